# revision 42
# baseline (speedup 1.0000x reference)
"""Trainium2 Bass kernel: single-head attention with QKV projections.

Problem (hardcoded): q/k/v [4,2048,1024] fp32, W_q/W_k/W_v [1024,1024] fp32;
out = softmax((x@Wq^T)(x@Wk^T)^T/32) @ (x@Wv^T), fp32 [4,2048,1024].

Sharding: 8 cores = 4 batches x 2 query-halves; no collectives.

Algebraic folding (host-side, weight-only):
  M = Wq^T @ Wk / 32  =>  sim = Xq @ M @ Xk^T   (K projection eliminated)
  out = (P @ Xv) @ Wv^T / rowsum(P)             (V projection reordered)
so each core consumes raw full-batch Xk/Xv directly and the per-core
matmul work drops from 7.5 GMAC to 6.4 GMAC with zero communication.

Phases per core (PSUM accumulation fp32):
  A: Q'^T = (M*SQ)^T Xq^T          bf16, 128 N=512 matmuls
  D: S^T  = Xk8 Q'8^T              fp8e4 DoubleRow (K=256/instr), 128 matmuls
     P^T  = exp(S^T/(SQ*SK))       scalar engine, stored bf16
  E: U^T  = Xv^T P^T               bf16, 256 N=512 matmuls
  F: out  = (U^T)^T Wv^T * r       bf16, 128 N=512 matmuls

Schedule notes (each worth real microseconds on HW):
- The lead-in DMA burst (m+xq, 4MB) is device-HBM-bound with all 8 cores
  pulling at once; it is split into column-range priority sets matching
  the phase-A passes, bulk tensors are deferred out of the burst via
  memset-anchored writer-after-writer deps, and phase A pass 1 runs
  dt-MAJOR over 6 half-width PSUM chains so matmuls consume each tile
  the moment it lands.
- Junk matmuls warm the PE HAM clock gate during the DMA lead-in.
- The fp8 cast of Q' runs on the vector engine (scale folded into M on
  host) so no compute op ever queues behind a blocked DMA trigger.
- The softmax denominator never touches the tensor engine: vector-engine
  adds accumulate sum_kt P^T during phase D, gpsimd partition_all_reduce
  finishes the k-reduction, tiny DMAs scatter the row into [128,8]
  per-partition layout (emitted after phase E so their queue-blocking
  waits cannot stall E), one reciprocal feeds phase F's output scaling.
- Output is written bf16 (upcast on host) in 256-col chunks alternating
  vector/scalar scaling and both output rings, halving the drain tail.
"""

import numpy as np
import ml_dtypes

P = 128
D = 1024          # d_model / contraction dims
QL = 1024         # queries per core (half batch)
KL = 2048         # keys per core (full batch)
DT = D // P       # 8 d-tiles
KT = KL // P      # 16 key tiles
QT = QL // P      # 8 query tiles
NG = D // 256     # 4 DoubleRow groups

SQ = 16.0         # fp8 scale on Q' (folded into M on host)
SK = 4.0          # fp8 scale on Xk (applied on host)
EXP_SCALE = 1.0 / (SQ * SK)
N_WARM = 10       # junk matmuls to warm the PE clock during DMA lead-in

_CACHE = {}


def _build_nc():
    from contextlib import ExitStack

    import concourse.bass as bass
    import concourse.mybir as mybir
    import concourse.tile as tile
    from concourse import bacc, bass_isa

    BF = mybir.dt.bfloat16
    F32 = mybir.dt.float32
    FP8 = mybir.dt.float8e4
    AFT = mybir.ActivationFunctionType
    DR = mybir.MatmulPerfMode.DoubleRow

    nc = bacc.Bacc("TRN2", target_bir_lowering=False, debug=False,
                   enable_asserts=False, num_devices=8)

    m_in = nc.dram_tensor("m_in", [D, D], BF, kind="ExternalInput").ap()
    xqT = nc.dram_tensor("xqT", [D, QL], BF, kind="ExternalInput").ap()
    xk8T = nc.dram_tensor("xk8T", [D, KL], FP8, kind="ExternalInput").ap()
    xv_in = nc.dram_tensor("xv_in", [KL, D], BF, kind="ExternalInput").ap()
    wvT = nc.dram_tensor("wvT", [D, D], BF, kind="ExternalInput").ap()
    out = nc.dram_tensor("out", [QL, D], BF, kind="ExternalOutput").ap()

    def r3(t, lo, n):
        return t[bass.ds(lo * P, n * P), :].rearrange("(t p) c -> p t c", p=P)

    with tile.TileContext(nc) as tc, ExitStack() as ctx:
        m_pool = ctx.enter_context(tc.tile_pool(name="m", bufs=1))
        xq_pool = ctx.enter_context(tc.tile_pool(name="xq", bufs=1))
        xk_pool = ctx.enter_context(tc.tile_pool(name="xk", bufs=1))
        xv_pool = ctx.enter_context(tc.tile_pool(name="xv", bufs=1))
        wv_pool = ctx.enter_context(tc.tile_pool(name="wv", bufs=1))
        q8_pool = ctx.enter_context(tc.tile_pool(name="q8", bufs=1))
        pt_pool = ctx.enter_context(tc.tile_pool(name="pT", bufs=1))
        ut_pool = ctx.enter_context(tc.tile_pool(name="uT", bufs=1))
        o_pool = ctx.enter_context(tc.tile_pool(name="o", bufs=3))
        small = ctx.enter_context(tc.tile_pool(name="small", bufs=1))
        spool = ctx.enter_context(tc.tile_pool(name="s", bufs=1))
        ps = ctx.enter_context(tc.tile_pool(name="ps", bufs=3, space="PSUM"))

        ones_t = small.tile([P, 1], BF, tag="ones")
        nc.vector.memset(ones_t, 1.0)
        junk_t = small.tile([P, 512], BF, tag="junk")
        nc.vector.memset(junk_t, 0.5)

        m_sb = m_pool.tile([P, DT, D], BF, tag="m")
        xq_sb = xq_pool.tile([P, DT, QL], BF, tag="xq")
        xk8_sb = xk_pool.tile([P, DT, KL], FP8, tag="xk8")
        xv_sb = xv_pool.tile([P, KT, D], BF, tag="xv")
        wv_sb = wv_pool.tile([P, DT, D], BF, tag="wv")
        q8_sb = q8_pool.tile([P, DT, QL], FP8, tag="q8")
        pT_sb = pt_pool.tile([P, KT, QL], BF, tag="pT")
        uT_sb = ut_pool.tile([P, DT, QL], BF, tag="uT")

        # ---- input DMAs ----
        # The lead-in DMA burst is HBM-bound (all 8 cores pull their 4MB
        # of m+xq simultaneously), so it is split by COLUMN RANGE into
        # three priority sets matching what each phase-A pass touches:
        #   set1: m cols 0:768 + xq cols 0:512   (pass 1: ets 0-5, c=0)
        #   set2: xq cols 512:1024               (pass 2: ets 0-5, c=1)
        #   set3: m cols 768:1024                (pass 3: ets 6-7)
        # so compute starts ~2.5MB into the burst instead of after 4MB.
        rings = [nc.sync, nc.scalar, nc.gpsimd]
        i = 0

        def lead_dma(sb, j, c0, c1, dram):
            nonlocal i
            rings[i % 3].dma_start(out=sb[:, j, c0:c1],
                                   in_=dram[j * P:(j + 1) * P, c0:c1])
            i += 1

        for dt in range(DT):
            lead_dma(m_sb, dt, 0, 768, m_in)
            lead_dma(xq_sb, dt, 0, 512, xqT)
        for dt in range(DT):
            lead_dma(xq_sb, dt, 512, 1024, xqT)
        for dt in range(DT):
            lead_dma(m_sb, dt, 768, 1024, m_in)
        # Bulk tensors are DEFERRED out of the lead-in burst: a 1-element
        # memset across a bulk tile's slices (on the vector queue, after a
        # given phase-A pass) makes the DMA triggers wait via the
        # writer-after-writer dependency; triggers are emitted inside the
        # phase-A loop below.

        # ---- PE warm-up: junk matmuls while the lead-in DMA lands ----
        junk_acc = ps.tile([P, QL], F32, tag="ps")
        for _ in range(N_WARM):
            nc.tensor.matmul(junk_acc[0:1, 0:512], ones_t[:, 0:1], junk_t,
                             start=True, stop=True)

        # ---- Phase A: Q'^T*SQ = (M*SQ)^T Xq^T, cast to fp8 on vector ----
        # Three passes of <=6 independent half-width (N=512) accumulation
        # chains packed into the 3 cycling PSUM bufs (2 chains per [P,1024]
        # tile). Pass 1 runs dt-MAJOR so each (m,xq) dt tile-pair is
        # consumed the moment its DMA lands — with et-major chains, no
        # chain could finish before the LAST lead tile landed and the
        # whole 27us of phase A serialized after the DMA window.
        # Pass sizes [4,4,4,4] (2 tiles each, bufs=3 cycling) make EVERY
        # pass-boundary buffer reuse land on a buffer freed a full pass
        # earlier, so no pass ever opens by waiting on the previous pass's
        # casts.
        passes = [
            [(0, 0), (1, 0), (2, 0), (3, 0)],
            [(4, 0), (5, 0), (0, 1), (1, 1)],
            [(2, 1), (3, 1), (4, 1), (5, 1)],
            [(6, 0), (7, 0), (6, 1), (7, 1)],
        ]
        for pi, chains in enumerate(passes):
            accs = [ps.tile([P, QL], F32, tag="ps", name=f"accA{pi}_{t}")
                    for t in range((len(chains) + 1) // 2)]
            sls = [accs[i // 2][:, (i % 2) * 512:(i % 2) * 512 + 512]
                   for i in range(len(chains))]
            for dt in range(DT):
                for i, (et, c) in enumerate(chains):
                    nc.tensor.matmul(
                        sls[i], m_sb[:, dt, et * P:(et + 1) * P],
                        xq_sb[:, dt, c * 512:(c + 1) * 512],
                        start=(dt == 0), stop=(dt == DT - 1))
            for i, (et, c) in enumerate(chains):
                nc.vector.tensor_copy(
                    q8_sb[:, et, c * 512:(c + 1) * 512], sls[i])
            # deferred bulk DMAs, anchored on this pass's first cast via a
            # writer-after-writer dep from a memset on the vector queue
            # (pass 1 ends while the lead-in sets 2-3 are still landing,
            # so bulk anchors start at pass 2)
            if pi == 1:
                nc.vector.memset(xk8_sb[0:1, :, 0:1], 0)
                nc.sync.dma_start(out=xk8_sb[:, 0:2, :], in_=r3(xk8T, 0, 2))
                nc.scalar.dma_start(out=xk8_sb[:, 2:4, :], in_=r3(xk8T, 2, 2))
                nc.gpsimd.dma_start(out=xk8_sb[:, 4:6, :], in_=r3(xk8T, 4, 2))
                nc.gpsimd.dma_start(out=xk8_sb[:, 6:8, :], in_=r3(xk8T, 6, 2))
            elif pi == 2:
                nc.vector.memset(xv_sb[0:1, :, 0:1], 0)
                for j in range(4):
                    eng = nc.sync if j % 2 == 0 else nc.scalar
                    eng.dma_start(out=xv_sb[:, 4 * j:4 * j + 4, :],
                                  in_=r3(xv_in, 4 * j, 4))
            elif pi == 3:
                nc.vector.memset(wv_sb[0:1, :, 0:1], 0)
                for j in range(2):
                    nc.gpsimd.dma_start(out=wv_sb[:, 4 * j:4 * j + 4, :],
                                        in_=r3(wvT, 4 * j, 4))
            # (pass 4's chains need m cols 768:1024 — lead set3 — so the
            # bulk never competes with a set the current pass is consuming)

        # ---- Phase D: S^T = Xk8^T-slices @ Q'8 via fp8 DoubleRow ----
        # The softmax denominator partials accumulate on the (otherwise
        # idle) vector engine as each exp tile lands, keeping the N=1
        # denominator matmuls off the tensor engine entirely.
        s_part = spool.tile([P, QL], F32, tag="s_part")
        for kt in range(KT):
            acc = ps.tile([P, QL], F32, tag="ps")
            for g in range(NG):
                k_sl = xk8_sb[:, 2 * g:2 * g + 2, kt * P:(kt + 1) * P]
                for c in range(2):
                    nc.tensor.matmul(
                        acc[:, c * 512:(c + 1) * 512], k_sl,
                        q8_sb[:, 2 * g:2 * g + 2, c * 512:(c + 1) * 512],
                        start=(g == 0), stop=(g == NG - 1),
                        perf_mode=DR)
            nc.scalar.activation(pT_sb[:, kt, :], acc, AFT.Exp,
                                 scale=EXP_SCALE)
            if kt == 0:
                nc.vector.tensor_copy(s_part, pT_sb[:, 0, :])
            else:
                nc.vector.tensor_add(s_part, s_part, pT_sb[:, kt, :])
        # finish the partition reduction on gpsimd (off the critical path)
        s_bc = spool.tile([P, QL], F32, tag="s_bc")
        nc.gpsimd.partition_all_reduce(s_bc, s_part, 128,
                                       bass_isa.ReduceOp.add)

        # ---- Phase E: U^T = Xv^T P^T ----
        for db in range(DT):
            acc = ps.tile([P, QL], F32, tag="ps")
            for kt in range(KT):
                v_sl = xv_sb[:, kt, db * P:(db + 1) * P]
                for c in range(2):
                    nc.tensor.matmul(
                        acc[:, c * 512:(c + 1) * 512], v_sl,
                        pT_sb[:, kt, c * 512:(c + 1) * 512],
                        start=(kt == 0), stop=(kt == KT - 1))
            nc.vector.tensor_copy(uT_sb[:, db, :], acc)

        # scatter s_bc's [1,1024] row into per-partition [128,8] layout and
        # take the reciprocal; emitted AFTER phase E so the queue-blocking
        # waits (on the gpsimd reduce) never stall E's copy chain
        s_cols = spool.tile([P, QT], F32, tag="s_cols")
        for t in range(QT):
            eng = nc.sync if t % 2 == 0 else nc.scalar
            eng.dma_start(out=s_cols[:, t:t + 1],
                          in_=s_bc[0:1, t * P:(t + 1) * P])
        r_all = spool.tile([P, QT], F32, tag="r_all")
        nc.vector.reciprocal(r_all, s_cols)

        # ---- Phase F: out = U Wv^T * r ----
        for qt in range(QT):
            acc = ps.tile([P, D], F32, tag="ps")
            if qt < QT - 1:
                for db in range(DT):
                    u_sl = uT_sb[:, db, qt * P:(qt + 1) * P]
                    for c in range(2):
                        nc.tensor.matmul(
                            acc[:, c * 512:(c + 1) * 512], u_sl,
                            wv_sb[:, db, c * 512:(c + 1) * 512],
                            start=(db == 0), stop=(db == DT - 1))
            else:
                # last tile: run the bank-B chain to completion FIRST so
                # its (longer) scalar scale+store drains under the bank-A
                # chain's matmuls — only the short vector path stays
                # exposed after the final matmul
                for c in (1, 0):
                    for db in range(DT):
                        u_sl = uT_sb[:, db, qt * P:(qt + 1) * P]
                        nc.tensor.matmul(
                            acc[:, c * 512:(c + 1) * 512], u_sl,
                            wv_sb[:, db, c * 512:(c + 1) * 512],
                            start=(db == 0), stop=(db == DT - 1))
            # output scaling: one PSUM BANK per engine (vector reads cols
            # 0:512 = bank A, scalar reads 512:1024 = bank B) into separate
            # staging tiles — any finer interleave serializes through the
            # bank-aware cross-engine PSUM-collision tracking
            r_t = r_all[:, qt:qt + 1]
            o_v = o_pool.tile([P, 512], BF, tag="ov", name=f"ov{qt}")
            o_s = o_pool.tile([P, 512], BF, tag="os", name=f"os{qt}")
            if qt == QT - 1:
                nc.scalar.activation(o_s, acc[:, 512:1024], AFT.Copy,
                                     scale=r_t)
                nc.scalar.dma_start(out=out[qt * P:(qt + 1) * P, 512:1024],
                                    in_=o_s)
            nc.vector.tensor_scalar_mul(o_v, acc[:, 0:512], r_t)
            if qt < QT - 1:
                nc.sync.dma_start(out=out[qt * P:(qt + 1) * P, 0:512],
                                  in_=o_v)
            else:
                # last tile: drain latency is exposed — use all 3 rings
                nc.sync.dma_start(out=out[qt * P:(qt + 1) * P, 0:256],
                                  in_=o_v[:, 0:256])
                nc.gpsimd.dma_start(out=out[qt * P:(qt + 1) * P, 256:512],
                                    in_=o_v[:, 256:512])
            if qt < QT - 1:
                nc.scalar.activation(o_s, acc[:, 512:1024], AFT.Copy,
                                     scale=r_t)
                nc.scalar.dma_start(out=out[qt * P:(qt + 1) * P, 512:1024],
                                    in_=o_s)

    nc.compile()
    return nc


def _get_nc():
    if "nc" not in _CACHE:
        _CACHE["nc"] = _build_nc()
    return _CACHE["nc"]


def make_in_maps(q, k, v, W_q, W_k, W_v):
    bf = ml_dtypes.bfloat16
    f8 = ml_dtypes.float8_e4m3
    W_q = np.asarray(W_q, dtype=np.float32)
    W_k = np.asarray(W_k, dtype=np.float32)
    W_v = np.asarray(W_v, dtype=np.float32)
    m_host = ((W_q.T @ W_k) * (SQ / 32.0)).astype(bf)
    wvT_host = np.ascontiguousarray(W_v.T).astype(bf)
    in_maps = []
    for c in range(8):
        b, h = c // 2, c % 2
        sl = slice(h * 1024, (h + 1) * 1024)
        in_maps.append({
            "m_in": m_host,
            "xqT": np.asarray(q[b, sl, :], dtype=np.float32).T.astype(bf),
            "xk8T": (np.asarray(k[b], dtype=np.float32).T * SK).astype(f8),
            "xv_in": np.asarray(v[b], dtype=np.float32).astype(bf),
            "wvT": wvT_host,
        })
    return in_maps


def kernel(**inputs):
    from concourse import bass_utils

    q = np.asarray(inputs["q_input"], dtype=np.float32)
    k = np.asarray(inputs["k_input"], dtype=np.float32)
    v = np.asarray(inputs["v_input"], dtype=np.float32)

    nc = _get_nc()
    in_maps = make_in_maps(q, k, v, inputs["W_q"], inputs["W_k"], inputs["W_v"])

    res = None
    for attempt in range(3):
        try:
            res = bass_utils.run_bass_kernel_spmd(nc, in_maps,
                                                  core_ids=list(range(8)))
            break
        except Exception:
            if attempt == 2:
                raise

    full = np.empty((4, 2048, 1024), dtype=np.float32)
    for c in range(8):
        b, h = c // 2, c % 2
        full[b, h * 1024:(h + 1) * 1024, :] = np.asarray(
            res.results[c]["out"], dtype=np.float32)
    return full


# revision 43
# speedup vs baseline: 1.0094x; 1.0094x over previous
"""Trainium2 Bass kernel: single-head attention with QKV projections.

Problem (hardcoded): q/k/v [4,2048,1024] fp32, W_q/W_k/W_v [1024,1024] fp32;
out = softmax((x@Wq^T)(x@Wk^T)^T/32) @ (x@Wv^T), fp32 [4,2048,1024].

Sharding: 8 cores = 4 batches x 2 query-halves; no collectives.

Algebraic folding (host-side, weight-only):
  M = Wq^T @ Wk / 32  =>  sim = Xq @ M @ Xk^T   (K projection eliminated)
  out = (P @ Xv) @ Wv^T / rowsum(P)             (V projection reordered)
so each core consumes raw full-batch Xk/Xv directly and the per-core
matmul work drops from 7.5 GMAC to 6.4 GMAC with zero communication.

Phases per core (PSUM accumulation fp32):
  A: Q'^T = (M*SQ)^T Xq^T          bf16, 128 N=512 matmuls
  D: S^T  = Xk8 Q'8^T              fp8e4 DoubleRow (K=256/instr), 128 matmuls
     P^T  = exp(S^T/(SQ*SK))       scalar engine, stored bf16
  E: U^T  = Xv^T P^T               bf16, 256 N=512 matmuls
  F: out  = (U^T)^T Wv^T * r       bf16, 128 N=512 matmuls

Schedule notes (each worth real microseconds on HW):
- The lead-in DMA burst (m+xq, 4MB) is device-HBM-bound with all 8 cores
  pulling at once; it is split into column-range priority sets matching
  the phase-A passes, bulk tensors are deferred out of the burst via
  memset-anchored writer-after-writer deps, and phase A pass 1 runs
  dt-MAJOR over 6 half-width PSUM chains so matmuls consume each tile
  the moment it lands.
- Junk matmuls warm the PE HAM clock gate during the DMA lead-in.
- The fp8 cast of Q' runs on the vector engine (scale folded into M on
  host) so no compute op ever queues behind a blocked DMA trigger.
- The softmax denominator never touches the tensor engine: vector-engine
  adds accumulate sum_kt P^T during phase D, gpsimd partition_all_reduce
  finishes the k-reduction, tiny DMAs scatter the row into [128,8]
  per-partition layout (emitted after phase E so their queue-blocking
  waits cannot stall E), one reciprocal feeds phase F's output scaling.
- Output is written bf16 (upcast on host) in 256-col chunks alternating
  vector/scalar scaling and both output rings, halving the drain tail.
"""

import numpy as np
import ml_dtypes

P = 128
D = 1024          # d_model / contraction dims
QL = 1024         # queries per core (half batch)
KL = 2048         # keys per core (full batch)
DT = D // P       # 8 d-tiles
KT = KL // P      # 16 key tiles
QT = QL // P      # 8 query tiles
NG = D // 256     # 4 DoubleRow groups

SQ = 16.0         # fp8 scale on Q' (folded into M on host)
SK = 4.0          # fp8 scale on Xk (applied on host)
EXP_SCALE = 1.0 / (SQ * SK)
N_WARM = 10       # junk matmuls to warm the PE clock during DMA lead-in

_CACHE = {}


def _build_nc():
    from contextlib import ExitStack

    import concourse.bass as bass
    import concourse.mybir as mybir
    import concourse.tile as tile
    from concourse import bacc, bass_isa

    BF = mybir.dt.bfloat16
    F32 = mybir.dt.float32
    FP8 = mybir.dt.float8e4
    AFT = mybir.ActivationFunctionType
    DR = mybir.MatmulPerfMode.DoubleRow

    nc = bacc.Bacc("TRN2", target_bir_lowering=False, debug=False,
                   enable_asserts=False, num_devices=8)

    m_in = nc.dram_tensor("m_in", [D, D], BF, kind="ExternalInput").ap()
    xqT = nc.dram_tensor("xqT", [D, QL], BF, kind="ExternalInput").ap()
    xk8T = nc.dram_tensor("xk8T", [D, KL], FP8, kind="ExternalInput").ap()
    xv_in = nc.dram_tensor("xv_in", [KL, D], BF, kind="ExternalInput").ap()
    wvT = nc.dram_tensor("wvT", [D, D], BF, kind="ExternalInput").ap()
    out = nc.dram_tensor("out", [QL, D], BF, kind="ExternalOutput").ap()

    def r3(t, lo, n):
        return t[bass.ds(lo * P, n * P), :].rearrange("(t p) c -> p t c", p=P)

    with tile.TileContext(nc) as tc, ExitStack() as ctx:
        m_pool = ctx.enter_context(tc.tile_pool(name="m", bufs=1))
        xq_pool = ctx.enter_context(tc.tile_pool(name="xq", bufs=1))
        xk_pool = ctx.enter_context(tc.tile_pool(name="xk", bufs=1))
        xv_pool = ctx.enter_context(tc.tile_pool(name="xv", bufs=1))
        wv_pool = ctx.enter_context(tc.tile_pool(name="wv", bufs=1))
        q8_pool = ctx.enter_context(tc.tile_pool(name="q8", bufs=1))
        pt_pool = ctx.enter_context(tc.tile_pool(name="pT", bufs=1))
        ut_pool = ctx.enter_context(tc.tile_pool(name="uT", bufs=1))
        o_pool = ctx.enter_context(tc.tile_pool(name="o", bufs=3))
        small = ctx.enter_context(tc.tile_pool(name="small", bufs=1))
        spool = ctx.enter_context(tc.tile_pool(name="s", bufs=1))
        ps = ctx.enter_context(tc.tile_pool(name="ps", bufs=3, space="PSUM"))

        ones_t = small.tile([P, 1], BF, tag="ones")
        nc.vector.memset(ones_t, 1.0)
        junk_t = small.tile([P, 512], BF, tag="junk")
        nc.vector.memset(junk_t, 0.5)

        m_sb = m_pool.tile([P, DT, D], BF, tag="m")
        xq_sb = xq_pool.tile([P, DT, QL], BF, tag="xq")
        xk8_sb = xk_pool.tile([P, DT, KL], FP8, tag="xk8")
        xv_sb = xv_pool.tile([P, KT, D], BF, tag="xv")
        wv_sb = wv_pool.tile([P, DT, D], BF, tag="wv")
        q8_sb = q8_pool.tile([P, DT, QL], FP8, tag="q8")
        pT_sb = pt_pool.tile([P, KT, QL], BF, tag="pT")
        uT_sb = ut_pool.tile([P, DT, QL], BF, tag="uT")

        # ---- input DMAs ----
        # The lead-in DMA burst is HBM-bound (all 8 cores pull their 4MB
        # of m+xq simultaneously), so it is split by COLUMN RANGE into
        # three priority sets matching what each phase-A pass touches:
        #   set1: m cols 0:768 + xq cols 0:512   (pass 1: ets 0-5, c=0)
        #   set2: xq cols 512:1024               (pass 2: ets 0-5, c=1)
        #   set3: m cols 768:1024                (pass 3: ets 6-7)
        # so compute starts ~2.5MB into the burst instead of after 4MB.
        rings = [nc.sync, nc.scalar, nc.gpsimd]
        i = 0

        def lead_dma(sb, j, c0, c1, dram):
            nonlocal i
            rings[i % 3].dma_start(out=sb[:, j, c0:c1],
                                   in_=dram[j * P:(j + 1) * P, c0:c1])
            i += 1

        for dt in range(DT):
            lead_dma(m_sb, dt, 0, 768, m_in)
            lead_dma(xq_sb, dt, 0, 512, xqT)
        for dt in range(DT):
            lead_dma(xq_sb, dt, 512, 1024, xqT)
        for dt in range(DT):
            lead_dma(m_sb, dt, 768, 1024, m_in)
        # Bulk tensors are DEFERRED out of the lead-in burst: a 1-element
        # memset across a bulk tile's slices (on the vector queue, after a
        # given phase-A pass) makes the DMA triggers wait via the
        # writer-after-writer dependency; triggers are emitted inside the
        # phase-A loop below.

        # ---- PE warm-up: junk matmuls while the lead-in DMA lands ----
        junk_acc = ps.tile([P, QL], F32, tag="ps")
        for _ in range(N_WARM):
            nc.tensor.matmul(junk_acc[0:1, 0:512], ones_t[:, 0:1], junk_t,
                             start=True, stop=True)

        # ---- Phase A: Q'^T*SQ = (M*SQ)^T Xq^T, cast to fp8 on vector ----
        # Three passes of <=6 independent half-width (N=512) accumulation
        # chains packed into the 3 cycling PSUM bufs (2 chains per [P,1024]
        # tile). Pass 1 runs dt-MAJOR so each (m,xq) dt tile-pair is
        # consumed the moment its DMA lands — with et-major chains, no
        # chain could finish before the LAST lead tile landed and the
        # whole 27us of phase A serialized after the DMA window.
        # Pass sizes [4,4,4,4] (2 tiles each, bufs=3 cycling) make EVERY
        # pass-boundary buffer reuse land on a buffer freed a full pass
        # earlier, so no pass ever opens by waiting on the previous pass's
        # casts.
        passes = [
            [(0, 0), (1, 0), (2, 0), (3, 0)],
            [(4, 0), (5, 0), (0, 1), (1, 1)],
            [(2, 1), (3, 1), (4, 1), (5, 1)],
            [(6, 0), (7, 0), (6, 1), (7, 1)],
        ]
        for pi, chains in enumerate(passes):
            accs = [ps.tile([P, QL], F32, tag="ps", name=f"accA{pi}_{t}")
                    for t in range((len(chains) + 1) // 2)]
            sls = [accs[i // 2][:, (i % 2) * 512:(i % 2) * 512 + 512]
                   for i in range(len(chains))]
            for dt in range(DT):
                for i, (et, c) in enumerate(chains):
                    nc.tensor.matmul(
                        sls[i], m_sb[:, dt, et * P:(et + 1) * P],
                        xq_sb[:, dt, c * 512:(c + 1) * 512],
                        start=(dt == 0), stop=(dt == DT - 1))
            for i, (et, c) in enumerate(chains):
                nc.vector.tensor_copy(
                    q8_sb[:, et, c * 512:(c + 1) * 512], sls[i])
            # deferred bulk DMAs, anchored on this pass's first cast via a
            # writer-after-writer dep from a memset on the vector queue
            # (pass 1 ends while the lead-in sets 2-3 are still landing,
            # so bulk anchors start at pass 2)
            if pi == 1:
                nc.vector.memset(xk8_sb[0:1, :, 0:1], 0)
                nc.sync.dma_start(out=xk8_sb[:, 0:2, :], in_=r3(xk8T, 0, 2))
                nc.scalar.dma_start(out=xk8_sb[:, 2:4, :], in_=r3(xk8T, 2, 2))
                nc.gpsimd.dma_start(out=xk8_sb[:, 4:6, :], in_=r3(xk8T, 4, 2))
                nc.gpsimd.dma_start(out=xk8_sb[:, 6:8, :], in_=r3(xk8T, 6, 2))
            elif pi == 2:
                nc.vector.memset(xv_sb[0:1, :, 0:1], 0)
                for j in range(4):
                    eng = nc.sync if j % 2 == 0 else nc.scalar
                    eng.dma_start(out=xv_sb[:, 4 * j:4 * j + 4, :],
                                  in_=r3(xv_in, 4 * j, 4))
            elif pi == 3:
                nc.vector.memset(wv_sb[0:1, :, 0:1], 0)
                for j in range(2):
                    nc.gpsimd.dma_start(out=wv_sb[:, 4 * j:4 * j + 4, :],
                                        in_=r3(wvT, 4 * j, 4))
            # (pass 4's chains need m cols 768:1024 — lead set3 — so the
            # bulk never competes with a set the current pass is consuming)

        # ---- Phase D: S^T = Xk8^T-slices @ Q'8 via fp8 DoubleRow ----
        # The softmax denominator partials accumulate on the (otherwise
        # idle) vector engine as each exp tile lands, keeping the N=1
        # denominator matmuls off the tensor engine entirely.
        s_part = spool.tile([P, QL], F32, tag="s_part")
        for kt in range(KT):
            acc = ps.tile([P, QL], F32, tag="ps")
            for g in range(NG):
                k_sl = xk8_sb[:, 2 * g:2 * g + 2, kt * P:(kt + 1) * P]
                for c in range(2):
                    nc.tensor.matmul(
                        acc[:, c * 512:(c + 1) * 512], k_sl,
                        q8_sb[:, 2 * g:2 * g + 2, c * 512:(c + 1) * 512],
                        start=(g == 0), stop=(g == NG - 1),
                        perf_mode=DR)
            nc.scalar.activation(pT_sb[:, kt, :], acc, AFT.Exp,
                                 scale=EXP_SCALE)
            if kt == 0:
                nc.vector.tensor_copy(s_part, pT_sb[:, 0, :])
            else:
                nc.vector.tensor_add(s_part, s_part, pT_sb[:, kt, :])
        # finish the partition reduction on gpsimd (off the critical path)
        s_bc = spool.tile([P, QL], F32, tag="s_bc")
        nc.gpsimd.partition_all_reduce(s_bc, s_part, 128,
                                       bass_isa.ReduceOp.add)

        # ---- Phase E: U^T = Xv^T P^T ----
        for db in range(DT):
            acc = ps.tile([P, QL], F32, tag="ps")
            for kt in range(KT):
                v_sl = xv_sb[:, kt, db * P:(db + 1) * P]
                for c in range(2):
                    nc.tensor.matmul(
                        acc[:, c * 512:(c + 1) * 512], v_sl,
                        pT_sb[:, kt, c * 512:(c + 1) * 512],
                        start=(kt == 0), stop=(kt == KT - 1))
            nc.vector.tensor_copy(uT_sb[:, db, :], acc)

        # scatter s_bc's [1,1024] row into per-partition [128,8] layout and
        # take the reciprocal; emitted AFTER phase E so the queue-blocking
        # waits (on the gpsimd reduce) never stall E's copy chain
        s_cols = spool.tile([P, QT], F32, tag="s_cols")
        for t in range(QT):
            eng = nc.sync if t % 2 == 0 else nc.scalar
            eng.dma_start(out=s_cols[:, t:t + 1],
                          in_=s_bc[0:1, t * P:(t + 1) * P])
        r_all = spool.tile([P, QT], F32, tag="r_all")
        nc.vector.reciprocal(r_all, s_cols)

        # ---- Phase F: out = U Wv^T * r ----
        for qt in range(QT):
            acc = ps.tile([P, D], F32, tag="ps")
            for db in range(DT):
                u_sl = uT_sb[:, db, qt * P:(qt + 1) * P]
                for c in range(2):
                    nc.tensor.matmul(
                        acc[:, c * 512:(c + 1) * 512], u_sl,
                        wv_sb[:, db, c * 512:(c + 1) * 512],
                        start=(db == 0), stop=(db == DT - 1))
            # output scaling: one PSUM BANK per engine (vector reads cols
            # 0:512 = bank A, scalar reads 512:1024 = bank B) into separate
            # staging tiles — any finer interleave serializes through the
            # bank-aware cross-engine PSUM-collision tracking
            r_t = r_all[:, qt:qt + 1]
            o_v = o_pool.tile([P, 512], BF, tag="ov", name=f"ov{qt}")
            o_s = o_pool.tile([P, 512], BF, tag="os", name=f"os{qt}")
            nc.vector.tensor_scalar_mul(o_v, acc[:, 0:512], r_t)
            if qt < QT - 1:
                nc.sync.dma_start(out=out[qt * P:(qt + 1) * P, 0:512],
                                  in_=o_v)
            else:
                # last tile: drain latency is exposed — use all 3 rings
                nc.sync.dma_start(out=out[qt * P:(qt + 1) * P, 0:256],
                                  in_=o_v[:, 0:256])
                nc.gpsimd.dma_start(out=out[qt * P:(qt + 1) * P, 256:512],
                                    in_=o_v[:, 256:512])
            nc.scalar.activation(o_s, acc[:, 512:1024], AFT.Copy, scale=r_t)
            nc.scalar.dma_start(out=out[qt * P:(qt + 1) * P, 512:1024],
                                in_=o_s)

    nc.compile()
    return nc


def _get_nc():
    if "nc" not in _CACHE:
        _CACHE["nc"] = _build_nc()
    return _CACHE["nc"]


def make_in_maps(q, k, v, W_q, W_k, W_v):
    bf = ml_dtypes.bfloat16
    f8 = ml_dtypes.float8_e4m3
    W_q = np.asarray(W_q, dtype=np.float32)
    W_k = np.asarray(W_k, dtype=np.float32)
    W_v = np.asarray(W_v, dtype=np.float32)
    m_host = ((W_q.T @ W_k) * (SQ / 32.0)).astype(bf)
    wvT_host = np.ascontiguousarray(W_v.T).astype(bf)
    in_maps = []
    for c in range(8):
        b, h = c // 2, c % 2
        sl = slice(h * 1024, (h + 1) * 1024)
        in_maps.append({
            "m_in": m_host,
            "xqT": np.asarray(q[b, sl, :], dtype=np.float32).T.astype(bf),
            "xk8T": (np.asarray(k[b], dtype=np.float32).T * SK).astype(f8),
            "xv_in": np.asarray(v[b], dtype=np.float32).astype(bf),
            "wvT": wvT_host,
        })
    return in_maps


def kernel(**inputs):
    from concourse import bass_utils

    q = np.asarray(inputs["q_input"], dtype=np.float32)
    k = np.asarray(inputs["k_input"], dtype=np.float32)
    v = np.asarray(inputs["v_input"], dtype=np.float32)

    nc = _get_nc()
    in_maps = make_in_maps(q, k, v, inputs["W_q"], inputs["W_k"], inputs["W_v"])

    res = None
    for attempt in range(3):
        try:
            res = bass_utils.run_bass_kernel_spmd(nc, in_maps,
                                                  core_ids=list(range(8)))
            break
        except Exception:
            if attempt == 2:
                raise

    full = np.empty((4, 2048, 1024), dtype=np.float32)
    for c in range(8):
        b, h = c // 2, c % 2
        full[b, h * 1024:(h + 1) * 1024, :] = np.asarray(
            res.results[c]["out"], dtype=np.float32)
    return full


# revision 44
# speedup vs baseline: 1.0102x; 1.0008x over previous
"""Trainium2 Bass kernel: single-head attention with QKV projections.

Problem (hardcoded): q/k/v [4,2048,1024] fp32, W_q/W_k/W_v [1024,1024] fp32;
out = softmax((x@Wq^T)(x@Wk^T)^T/32) @ (x@Wv^T), fp32 [4,2048,1024].

Sharding: 8 cores = 4 batches x 2 query-halves; no collectives.

Algebraic folding (host-side, weight-only):
  M = Wq^T @ Wk / 32  =>  sim = Xq @ M @ Xk^T   (K projection eliminated)
  out = (P @ Xv) @ Wv^T / rowsum(P)             (V projection reordered)
so each core consumes raw full-batch Xk/Xv directly and the per-core
matmul work drops from 7.5 GMAC to 6.4 GMAC with zero communication.

Phases per core (PSUM accumulation fp32):
  A: Q'^T = (M*SQ)^T Xq^T          bf16, 128 N=512 matmuls
  D: S^T  = Xk8 Q'8^T              fp8e4 DoubleRow (K=256/instr), 128 matmuls
     P^T  = exp(S^T/(SQ*SK))       scalar engine, stored bf16
  E: U^T  = Xv^T P^T               bf16, 256 N=512 matmuls
  F: out  = (U^T)^T Wv^T * r       bf16, 128 N=512 matmuls

Schedule notes (each worth real microseconds on HW):
- The lead-in DMA burst (m+xq, 4MB) is device-HBM-bound with all 8 cores
  pulling at once; it is split into column-range priority sets matching
  the phase-A passes, bulk tensors are deferred out of the burst via
  memset-anchored writer-after-writer deps, and phase A pass 1 runs
  dt-MAJOR over 6 half-width PSUM chains so matmuls consume each tile
  the moment it lands.
- Junk matmuls warm the PE HAM clock gate during the DMA lead-in.
- The fp8 cast of Q' runs on the vector engine (scale folded into M on
  host) so no compute op ever queues behind a blocked DMA trigger.
- The softmax denominator never touches the tensor engine: vector-engine
  adds accumulate sum_kt P^T during phase D, gpsimd partition_all_reduce
  finishes the k-reduction, tiny DMAs scatter the row into [128,8]
  per-partition layout (emitted after phase E so their queue-blocking
  waits cannot stall E), one reciprocal feeds phase F's output scaling.
- Output is written bf16 (upcast on host) in 256-col chunks alternating
  vector/scalar scaling and both output rings, halving the drain tail.
"""

import numpy as np
import ml_dtypes

P = 128
D = 1024          # d_model / contraction dims
QL = 1024         # queries per core (half batch)
KL = 2048         # keys per core (full batch)
DT = D // P       # 8 d-tiles
KT = KL // P      # 16 key tiles
QT = QL // P      # 8 query tiles
NG = D // 256     # 4 DoubleRow groups

SQ = 16.0         # fp8 scale on Q' (folded into M on host)
SK = 4.0          # fp8 scale on Xk (applied on host)
EXP_SCALE = 1.0 / (SQ * SK)
N_WARM = 10       # junk matmuls to warm the PE clock during DMA lead-in

_CACHE = {}


def _build_nc():
    from contextlib import ExitStack

    import concourse.bass as bass
    import concourse.mybir as mybir
    import concourse.tile as tile
    from concourse import bacc, bass_isa

    BF = mybir.dt.bfloat16
    F32 = mybir.dt.float32
    FP8 = mybir.dt.float8e4
    AFT = mybir.ActivationFunctionType
    DR = mybir.MatmulPerfMode.DoubleRow

    nc = bacc.Bacc("TRN2", target_bir_lowering=False, debug=False,
                   enable_asserts=False, num_devices=8)

    m_in = nc.dram_tensor("m_in", [D, D], BF, kind="ExternalInput").ap()
    xqT = nc.dram_tensor("xqT", [D, QL], BF, kind="ExternalInput").ap()
    xk8T = nc.dram_tensor("xk8T", [D, KL], FP8, kind="ExternalInput").ap()
    xv_in = nc.dram_tensor("xv_in", [KL, D], BF, kind="ExternalInput").ap()
    wvT = nc.dram_tensor("wvT", [D, D], BF, kind="ExternalInput").ap()
    out = nc.dram_tensor("out", [QL, D], BF, kind="ExternalOutput").ap()

    def r3(t, lo, n):
        return t[bass.ds(lo * P, n * P), :].rearrange("(t p) c -> p t c", p=P)

    with tile.TileContext(nc) as tc, ExitStack() as ctx:
        m_pool = ctx.enter_context(tc.tile_pool(name="m", bufs=1))
        xq_pool = ctx.enter_context(tc.tile_pool(name="xq", bufs=1))
        xk_pool = ctx.enter_context(tc.tile_pool(name="xk", bufs=1))
        xv_pool = ctx.enter_context(tc.tile_pool(name="xv", bufs=1))
        wv_pool = ctx.enter_context(tc.tile_pool(name="wv", bufs=1))
        q8_pool = ctx.enter_context(tc.tile_pool(name="q8", bufs=1))
        pt_pool = ctx.enter_context(tc.tile_pool(name="pT", bufs=1))
        ut_pool = ctx.enter_context(tc.tile_pool(name="uT", bufs=1))
        o_pool = ctx.enter_context(tc.tile_pool(name="o", bufs=3))
        small = ctx.enter_context(tc.tile_pool(name="small", bufs=1))
        spool = ctx.enter_context(tc.tile_pool(name="s", bufs=1))
        ps = ctx.enter_context(tc.tile_pool(name="ps", bufs=3, space="PSUM"))

        ones_t = small.tile([P, 1], BF, tag="ones")
        nc.vector.memset(ones_t, 1.0)
        junk_t = small.tile([P, 512], BF, tag="junk")
        nc.vector.memset(junk_t, 0.5)

        m_sb = m_pool.tile([P, DT, D], BF, tag="m")
        xq_sb = xq_pool.tile([P, DT, QL], BF, tag="xq")
        xk8_sb = xk_pool.tile([P, DT, KL], FP8, tag="xk8")
        xv_sb = xv_pool.tile([P, KT, D], BF, tag="xv")
        wv_sb = wv_pool.tile([P, DT, D], BF, tag="wv")
        q8_sb = q8_pool.tile([P, DT, QL], FP8, tag="q8")
        pT_sb = pt_pool.tile([P, KT, QL], BF, tag="pT")
        uT_sb = ut_pool.tile([P, DT, QL], BF, tag="uT")

        # ---- input DMAs ----
        # The lead-in DMA burst is HBM-bound (all 8 cores pull their 4MB
        # of m+xq simultaneously), so it is split by COLUMN RANGE into
        # three priority sets matching what each phase-A pass touches:
        #   set1: m cols 0:768 + xq cols 0:512   (pass 1: ets 0-5, c=0)
        #   set2: xq cols 512:1024               (pass 2: ets 0-5, c=1)
        #   set3: m cols 768:1024                (pass 3: ets 6-7)
        # so compute starts ~2.5MB into the burst instead of after 4MB.
        rings = [nc.sync, nc.scalar, nc.gpsimd]
        i = 0

        def lead_dma(sb, j, c0, c1, dram):
            nonlocal i
            rings[i % 3].dma_start(out=sb[:, j, c0:c1],
                                   in_=dram[j * P:(j + 1) * P, c0:c1])
            i += 1

        for dt in range(DT):
            lead_dma(m_sb, dt, 0, 768, m_in)
            lead_dma(xq_sb, dt, 0, 512, xqT)
        for dt in range(DT):
            lead_dma(xq_sb, dt, 512, 1024, xqT)
        for dt in range(DT):
            lead_dma(m_sb, dt, 768, 1024, m_in)
        # Bulk tensors are DEFERRED out of the lead-in burst: a 1-element
        # memset across a bulk tile's slices (on the vector queue, after a
        # given phase-A pass) makes the DMA triggers wait via the
        # writer-after-writer dependency; triggers are emitted inside the
        # phase-A loop below.

        # ---- PE warm-up: junk matmuls while the lead-in DMA lands ----
        junk_acc = ps.tile([P, QL], F32, tag="ps")
        for _ in range(N_WARM):
            nc.tensor.matmul(junk_acc[0:1, 0:512], ones_t[:, 0:1], junk_t,
                             start=True, stop=True)

        # ---- Phase A: Q'^T*SQ = (M*SQ)^T Xq^T, cast to fp8 on vector ----
        # Three passes of <=6 independent half-width (N=512) accumulation
        # chains packed into the 3 cycling PSUM bufs (2 chains per [P,1024]
        # tile). Pass 1 runs dt-MAJOR so each (m,xq) dt tile-pair is
        # consumed the moment its DMA lands — with et-major chains, no
        # chain could finish before the LAST lead tile landed and the
        # whole 27us of phase A serialized after the DMA window.
        # Pass sizes [4,4,4,4] (2 tiles each, bufs=3 cycling) make EVERY
        # pass-boundary buffer reuse land on a buffer freed a full pass
        # earlier, so no pass ever opens by waiting on the previous pass's
        # casts.
        passes = [
            [(0, 0), (1, 0), (2, 0), (3, 0)],
            [(4, 0), (5, 0), (0, 1), (1, 1)],
            [(2, 1), (3, 1), (4, 1), (5, 1)],
            [(6, 0), (7, 0), (6, 1), (7, 1)],
        ]
        for pi, chains in enumerate(passes):
            accs = [ps.tile([P, QL], F32, tag="ps", name=f"accA{pi}_{t}")
                    for t in range((len(chains) + 1) // 2)]
            sls = [accs[i // 2][:, (i % 2) * 512:(i % 2) * 512 + 512]
                   for i in range(len(chains))]
            for dt in range(DT):
                for i, (et, c) in enumerate(chains):
                    nc.tensor.matmul(
                        sls[i], m_sb[:, dt, et * P:(et + 1) * P],
                        xq_sb[:, dt, c * 512:(c + 1) * 512],
                        start=(dt == 0), stop=(dt == DT - 1))
            for i, (et, c) in enumerate(chains):
                nc.vector.tensor_copy(
                    q8_sb[:, et, c * 512:(c + 1) * 512], sls[i])
            # deferred bulk DMAs, anchored on this pass's first cast via a
            # writer-after-writer dep from a memset on the vector queue
            # (pass 1 ends while the lead-in sets 2-3 are still landing,
            # so bulk anchors start at pass 2)
            if pi == 1:
                nc.vector.memset(xk8_sb[0:1, :, 0:1], 0)
                nc.sync.dma_start(out=xk8_sb[:, 0:2, :], in_=r3(xk8T, 0, 2))
                nc.scalar.dma_start(out=xk8_sb[:, 2:4, :], in_=r3(xk8T, 2, 2))
                nc.gpsimd.dma_start(out=xk8_sb[:, 4:6, :], in_=r3(xk8T, 4, 2))
                nc.gpsimd.dma_start(out=xk8_sb[:, 6:8, :], in_=r3(xk8T, 6, 2))
            elif pi == 2:
                nc.vector.memset(xv_sb[0:1, :, 0:1], 0)
                for j in range(4):
                    eng = nc.sync if j % 2 == 0 else nc.scalar
                    eng.dma_start(out=xv_sb[:, 4 * j:4 * j + 4, :],
                                  in_=r3(xv_in, 4 * j, 4))
            elif pi == 3:
                nc.vector.memset(wv_sb[0:1, :, 0:1], 0)
                for j in range(2):
                    nc.gpsimd.dma_start(out=wv_sb[:, 4 * j:4 * j + 4, :],
                                        in_=r3(wvT, 4 * j, 4))
            # (pass 4's chains need m cols 768:1024 — lead set3 — so the
            # bulk never competes with a set the current pass is consuming)

        # ---- Phase D: S^T = Xk8^T-slices @ Q'8 via fp8 DoubleRow ----
        # The softmax denominator partials accumulate on the (otherwise
        # idle) vector engine as each exp tile lands, keeping the N=1
        # denominator matmuls off the tensor engine entirely.
        s_part = spool.tile([P, QL], F32, tag="s_part")
        for kt in range(KT):
            acc = ps.tile([P, QL], F32, tag="ps")
            for g in range(NG):
                k_sl = xk8_sb[:, 2 * g:2 * g + 2, kt * P:(kt + 1) * P]
                for c in range(2):
                    nc.tensor.matmul(
                        acc[:, c * 512:(c + 1) * 512], k_sl,
                        q8_sb[:, 2 * g:2 * g + 2, c * 512:(c + 1) * 512],
                        start=(g == 0), stop=(g == NG - 1),
                        perf_mode=DR)
            nc.scalar.activation(pT_sb[:, kt, :], acc, AFT.Exp,
                                 scale=EXP_SCALE)
            if kt == 0:
                nc.vector.tensor_copy(s_part, pT_sb[:, 0, :])
            else:
                nc.vector.tensor_add(s_part, s_part, pT_sb[:, kt, :])
        # finish the partition reduction on gpsimd (off the critical path)
        s_bc = spool.tile([P, QL], F32, tag="s_bc")
        nc.gpsimd.partition_all_reduce(s_bc, s_part, 128,
                                       bass_isa.ReduceOp.add)

        # ---- Phase E: U^T = Xv^T P^T ----
        for db in range(DT):
            acc = ps.tile([P, QL], F32, tag="ps")
            for kt in range(KT):
                v_sl = xv_sb[:, kt, db * P:(db + 1) * P]
                for c in range(2):
                    nc.tensor.matmul(
                        acc[:, c * 512:(c + 1) * 512], v_sl,
                        pT_sb[:, kt, c * 512:(c + 1) * 512],
                        start=(kt == 0), stop=(kt == KT - 1))
            nc.vector.tensor_copy(uT_sb[:, db, :], acc)

        # scatter s_bc's [1,1024] row into per-partition [128,8] layout and
        # take the reciprocal; emitted AFTER phase E so the queue-blocking
        # waits (on the gpsimd reduce) never stall E's copy chain
        s_cols = spool.tile([P, QT], F32, tag="s_cols")
        for t in range(QT):
            eng = nc.sync if t % 2 == 0 else nc.scalar
            eng.dma_start(out=s_cols[:, t:t + 1],
                          in_=s_bc[0:1, t * P:(t + 1) * P])
        r_all = spool.tile([P, QT], F32, tag="r_all")
        nc.vector.reciprocal(r_all, s_cols)

        # ---- Phase F: out = U Wv^T * r ----
        for qt in range(QT):
            acc = ps.tile([P, D], F32, tag="ps")
            for db in range(DT):
                u_sl = uT_sb[:, db, qt * P:(qt + 1) * P]
                for c in range(2):
                    nc.tensor.matmul(
                        acc[:, c * 512:(c + 1) * 512], u_sl,
                        wv_sb[:, db, c * 512:(c + 1) * 512],
                        start=(db == 0), stop=(db == DT - 1))
            # output scaling: one PSUM BANK per engine (vector reads cols
            # 0:512 = bank A, scalar reads 512:1024 = bank B) into separate
            # staging tiles — any finer interleave serializes through the
            # bank-aware cross-engine PSUM-collision tracking
            r_t = r_all[:, qt:qt + 1]
            o_v = o_pool.tile([P, 512], BF, tag="ov", name=f"ov{qt}")
            o_s = o_pool.tile([P, 512], BF, tag="os", name=f"os{qt}")
            nc.vector.tensor_scalar_mul(o_v, acc[:, 0:512], r_t)
            if qt < QT - 1:
                nc.sync.dma_start(out=out[qt * P:(qt + 1) * P, 0:512],
                                  in_=o_v)
            else:
                # last tile: drain latency is exposed — use all 3 rings
                nc.sync.dma_start(out=out[qt * P:(qt + 1) * P, 0:256],
                                  in_=o_v[:, 0:256])
                nc.gpsimd.dma_start(out=out[qt * P:(qt + 1) * P, 256:512],
                                    in_=o_v[:, 256:512])
            nc.scalar.activation(o_s, acc[:, 512:1024], AFT.Copy, scale=r_t)
            if qt < QT - 1:
                nc.scalar.dma_start(out=out[qt * P:(qt + 1) * P, 512:1024],
                                    in_=o_s)
            else:
                # last tile: single act (two serialized acts regressed), but
                # split its store across two rings to halve the 128KB drain
                nc.scalar.dma_start(out=out[qt * P:(qt + 1) * P, 512:768],
                                    in_=o_s[:, 0:256])
                nc.sync.dma_start(out=out[qt * P:(qt + 1) * P, 768:1024],
                                  in_=o_s[:, 256:512])

    nc.compile()
    return nc


def _get_nc():
    if "nc" not in _CACHE:
        _CACHE["nc"] = _build_nc()
    return _CACHE["nc"]


def make_in_maps(q, k, v, W_q, W_k, W_v):
    bf = ml_dtypes.bfloat16
    f8 = ml_dtypes.float8_e4m3
    W_q = np.asarray(W_q, dtype=np.float32)
    W_k = np.asarray(W_k, dtype=np.float32)
    W_v = np.asarray(W_v, dtype=np.float32)
    m_host = ((W_q.T @ W_k) * (SQ / 32.0)).astype(bf)
    wvT_host = np.ascontiguousarray(W_v.T).astype(bf)
    in_maps = []
    for c in range(8):
        b, h = c // 2, c % 2
        sl = slice(h * 1024, (h + 1) * 1024)
        in_maps.append({
            "m_in": m_host,
            "xqT": np.asarray(q[b, sl, :], dtype=np.float32).T.astype(bf),
            "xk8T": (np.asarray(k[b], dtype=np.float32).T * SK).astype(f8),
            "xv_in": np.asarray(v[b], dtype=np.float32).astype(bf),
            "wvT": wvT_host,
        })
    return in_maps


def kernel(**inputs):
    from concourse import bass_utils

    q = np.asarray(inputs["q_input"], dtype=np.float32)
    k = np.asarray(inputs["k_input"], dtype=np.float32)
    v = np.asarray(inputs["v_input"], dtype=np.float32)

    nc = _get_nc()
    in_maps = make_in_maps(q, k, v, inputs["W_q"], inputs["W_k"], inputs["W_v"])

    res = None
    for attempt in range(3):
        try:
            res = bass_utils.run_bass_kernel_spmd(nc, in_maps,
                                                  core_ids=list(range(8)))
            break
        except Exception:
            if attempt == 2:
                raise

    full = np.empty((4, 2048, 1024), dtype=np.float32)
    for c in range(8):
        b, h = c // 2, c % 2
        full[b, h * 1024:(h + 1) * 1024, :] = np.asarray(
            res.results[c]["out"], dtype=np.float32)
    return full


# revision 47
# speedup vs baseline: 1.0374x; 1.0269x over previous
"""Trainium2 Bass kernel: single-head attention with QKV projections.

Problem (hardcoded): q/k/v [4,2048,1024] fp32, W_q/W_k/W_v [1024,1024] fp32;
out = softmax((x@Wq^T)(x@Wk^T)^T/32) @ (x@Wv^T), fp32 [4,2048,1024].

Sharding: 8 cores = 4 batches x 2 query-halves; no collectives.

Algebraic folding (host-side, weight-only):
  M = Wq^T @ Wk / 32  =>  sim = Xq @ M @ Xk^T   (K projection eliminated)
  out = (P @ Xv) @ Wv^T / rowsum(P)             (V projection reordered)
so each core consumes raw full-batch Xk/Xv directly and the per-core
matmul work drops from 7.5 GMAC to 6.4 GMAC with zero communication.

Phases per core (PSUM accumulation fp32):
  A: Q'^T = (M*SQ)^T Xq^T          bf16, 128 N=512 matmuls
  D: S^T  = Xk8 Q'8^T              fp8e4 DoubleRow (K=256/instr), 128 matmuls
     P^T  = exp(S^T/(SQ*SK))       scalar engine, stored bf16
  E: U^T  = Xv^T P^T               bf16, 256 N=512 matmuls
  F: out  = (U^T)^T Wv^T * r       bf16, 128 N=512 matmuls

Schedule notes (each worth real microseconds on HW):
- The lead-in DMA burst (m+xq, 4MB) is device-HBM-bound with all 8 cores
  pulling at once; it is split into column-range priority sets matching
  the phase-A passes, bulk tensors are deferred out of the burst via
  memset-anchored writer-after-writer deps, and phase A pass 1 runs
  dt-MAJOR over 6 half-width PSUM chains so matmuls consume each tile
  the moment it lands.
- Junk matmuls warm the PE HAM clock gate during the DMA lead-in.
- The fp8 cast of Q' runs on the vector engine (scale folded into M on
  host) so no compute op ever queues behind a blocked DMA trigger.
- The softmax denominator never touches the tensor engine: vector-engine
  adds accumulate sum_kt P^T during phase D, gpsimd partition_all_reduce
  finishes the k-reduction, tiny DMAs scatter the row into [128,8]
  per-partition layout (emitted after phase E so their queue-blocking
  waits cannot stall E), one reciprocal feeds phase F's output scaling.
- Output is written bf16 (upcast on host) in 256-col chunks alternating
  vector/scalar scaling and both output rings, halving the drain tail.
"""

import numpy as np
import ml_dtypes

P = 128
D = 1024          # d_model / contraction dims
QL = 1024         # queries per core (half batch)
KL = 2048         # keys per core (full batch)
DT = D // P       # 8 d-tiles
KT = KL // P      # 16 key tiles
QT = QL // P      # 8 query tiles
NG = D // 256     # 4 DoubleRow groups

SQ = 16.0         # fp8 scale on Q' (folded into M on host)
SM8 = 4096.0      # scale on M for phase A's fp8 half (bf16 half matches)
SK = 4.0          # fp8 scale on Xk (applied on host)
EXP_SCALE = 1.0 / (SQ * SK)
N_WARM = 10       # junk matmuls to warm the PE clock during DMA lead-in

_CACHE = {}


def _build_nc():
    from contextlib import ExitStack

    import concourse.bass as bass
    import concourse.mybir as mybir
    import concourse.tile as tile
    from concourse import bacc, bass_isa

    BF = mybir.dt.bfloat16
    F32 = mybir.dt.float32
    FP8 = mybir.dt.float8e4
    AFT = mybir.ActivationFunctionType
    DR = mybir.MatmulPerfMode.DoubleRow

    nc = bacc.Bacc("TRN2", target_bir_lowering=False, debug=False,
                   enable_asserts=False, num_devices=8)

    m_in = nc.dram_tensor("m_in", [D, D], BF, kind="ExternalInput").ap()
    xqT = nc.dram_tensor("xqT", [D, QL], BF, kind="ExternalInput").ap()
    m8T = nc.dram_tensor("m8T", [512, D], FP8, kind="ExternalInput").ap()
    xq8T = nc.dram_tensor("xq8T", [512, QL], FP8, kind="ExternalInput").ap()
    xk8T = nc.dram_tensor("xk8T", [D, KL], FP8, kind="ExternalInput").ap()
    xv_in = nc.dram_tensor("xv_in", [KL, D], BF, kind="ExternalInput").ap()
    wvT = nc.dram_tensor("wvT", [D, D], BF, kind="ExternalInput").ap()
    out = nc.dram_tensor("out", [QL, D], BF, kind="ExternalOutput").ap()

    def r3(t, lo, n):
        return t[bass.ds(lo * P, n * P), :].rearrange("(t p) c -> p t c", p=P)

    with tile.TileContext(nc) as tc, ExitStack() as ctx:
        m_pool = ctx.enter_context(tc.tile_pool(name="m", bufs=1))
        xq_pool = ctx.enter_context(tc.tile_pool(name="xq", bufs=1))
        xk_pool = ctx.enter_context(tc.tile_pool(name="xk", bufs=1))
        xv_pool = ctx.enter_context(tc.tile_pool(name="xv", bufs=1))
        wv_pool = ctx.enter_context(tc.tile_pool(name="wv", bufs=1))
        q8_pool = ctx.enter_context(tc.tile_pool(name="q8", bufs=1))
        pt_pool = ctx.enter_context(tc.tile_pool(name="pT", bufs=1))
        ut_pool = ctx.enter_context(tc.tile_pool(name="uT", bufs=1))
        o_pool = ctx.enter_context(tc.tile_pool(name="o", bufs=3))
        small = ctx.enter_context(tc.tile_pool(name="small", bufs=1))
        spool = ctx.enter_context(tc.tile_pool(name="s", bufs=1))
        ps = ctx.enter_context(tc.tile_pool(name="ps", bufs=3, space="PSUM"))

        ones_t = small.tile([P, 1], BF, tag="ones")
        nc.vector.memset(ones_t, 1.0)
        junk_t = small.tile([P, 512], BF, tag="junk")
        nc.vector.memset(junk_t, 0.5)

        m_sb = m_pool.tile([P, DT, D], BF, tag="m")
        xq_sb = xq_pool.tile([P, DT, QL], BF, tag="xq")
        m8_sb = m_pool.tile([P, 4, D], FP8, tag="m8")
        xq8_sb = xq_pool.tile([P, 4, QL], FP8, tag="xq8")
        xk8_sb = xk_pool.tile([P, DT, KL], FP8, tag="xk8")
        xv_sb = xv_pool.tile([P, KT, D], BF, tag="xv")
        wv_sb = wv_pool.tile([P, DT, D], BF, tag="wv")
        q8_sb = q8_pool.tile([P, DT, QL], FP8, tag="q8")
        pT_sb = pt_pool.tile([P, KT, QL], BF, tag="pT")
        uT_sb = ut_pool.tile([P, DT, QL], BF, tag="uT")

        # ---- input DMAs ----
        # The lead-in DMA burst is HBM-bound (all 8 cores pull their 4MB
        # of m+xq simultaneously), so it is split by COLUMN RANGE into
        # three priority sets matching what each phase-A pass touches:
        #   set1: m cols 0:768 + xq cols 0:512   (pass 1: ets 0-5, c=0)
        #   set2: xq cols 512:1024               (pass 2: ets 0-5, c=1)
        #   set3: m cols 768:1024                (pass 3: ets 6-7)
        # so compute starts ~2.5MB into the burst instead of after 4MB.
        rings = [nc.sync, nc.scalar, nc.gpsimd]
        i = 0

        def lead_dma(sb, j, c0, c1, dram):
            nonlocal i
            if sb is m8_sb or sb is xq8_sb:
                rings[i % 3].dma_start(out=sb[:, j:j + 2, :],
                                       in_=r3(dram, j, 2))
            else:
                rings[i % 3].dma_start(out=sb[:, j, c0:c1],
                                       in_=dram[j * P:(j + 1) * P, c0:c1])
            i += 1

        # set0: the fp8 half of phase A's contraction (1MB, lands first)
        for j in range(2):
            lead_dma(m8_sb, 2 * j, 0, D, m8T)      # [128,2,D] via j slicing
            lead_dma(xq8_sb, 2 * j, 0, QL, xq8T)
        for dt in range(4, DT):
            lead_dma(m_sb, dt, 0, 768, m_in)
            lead_dma(xq_sb, dt, 0, 512, xqT)
        for dt in range(4, DT):
            lead_dma(xq_sb, dt, 512, 1024, xqT)
        for dt in range(4, DT):
            lead_dma(m_sb, dt, 768, 1024, m_in)
        # Bulk tensors are DEFERRED out of the lead-in burst: a 1-element
        # memset across a bulk tile's slices (on the vector queue, after a
        # given phase-A pass) makes the DMA triggers wait via the
        # writer-after-writer dependency; triggers are emitted inside the
        # phase-A loop below.

        # ---- PE warm-up: junk matmuls while the lead-in DMA lands ----
        junk_acc = ps.tile([P, QL], F32, tag="ps")
        for _ in range(N_WARM):
            nc.tensor.matmul(junk_acc[0:1, 0:512], ones_t[:, 0:1], junk_t,
                             start=True, stop=True)

        # ---- Phase A: Q'^T*SQ = (M*SQ)^T Xq^T, cast to fp8 on vector ----
        # Three passes of <=6 independent half-width (N=512) accumulation
        # chains packed into the 3 cycling PSUM bufs (2 chains per [P,1024]
        # tile). Pass 1 runs dt-MAJOR so each (m,xq) dt tile-pair is
        # consumed the moment its DMA lands — with et-major chains, no
        # chain could finish before the LAST lead tile landed and the
        # whole 27us of phase A serialized after the DMA window.
        # Pass sizes [4,4,4,4] (2 tiles each, bufs=3 cycling) make EVERY
        # pass-boundary buffer reuse land on a buffer freed a full pass
        # earlier, so no pass ever opens by waiting on the previous pass's
        # casts.
        passes = [
            [(0, 0), (1, 0), (2, 0), (3, 0)],
            [(4, 0), (5, 0), (0, 1), (1, 1)],
            [(2, 1), (3, 1), (4, 1), (5, 1)],
            [(6, 0), (7, 0), (6, 1), (7, 1)],
        ]
        for pi, chains in enumerate(passes):
            accs = [ps.tile([P, QL], F32, tag="ps", name=f"accA{pi}_{t}")
                    for t in range((len(chains) + 1) // 2)]
            sls = [accs[i // 2][:, (i % 2) * 512:(i % 2) * 512 + 512]
                   for i in range(len(chains))]
            for g in range(2):
                for i, (et, c) in enumerate(chains):
                    nc.tensor.matmul(
                        sls[i],
                        m8_sb[:, 2 * g:2 * g + 2, et * P:(et + 1) * P],
                        xq8_sb[:, 2 * g:2 * g + 2, c * 512:(c + 1) * 512],
                        start=(g == 0), stop=False,
                        perf_mode=DR)
            for dt in range(4, DT):
                for i, (et, c) in enumerate(chains):
                    nc.tensor.matmul(
                        sls[i], m_sb[:, dt, et * P:(et + 1) * P],
                        xq_sb[:, dt, c * 512:(c + 1) * 512],
                        start=False, stop=(dt == DT - 1))
            for i, (et, c) in enumerate(chains):
                nc.vector.tensor_scalar_mul(
                    q8_sb[:, et, c * 512:(c + 1) * 512], sls[i],
                    float(SQ / SM8))
            # deferred bulk DMAs, anchored on this pass's first cast via a
            # writer-after-writer dep from a memset on the vector queue
            # (pass 1 ends while the lead-in sets 2-3 are still landing,
            # so bulk anchors start at pass 2)
            if pi == 1:
                nc.vector.memset(xk8_sb[0:1, :, 0:1], 0)
                nc.sync.dma_start(out=xk8_sb[:, 0:2, :], in_=r3(xk8T, 0, 2))
                nc.scalar.dma_start(out=xk8_sb[:, 2:4, :], in_=r3(xk8T, 2, 2))
                nc.gpsimd.dma_start(out=xk8_sb[:, 4:6, :], in_=r3(xk8T, 4, 2))
                nc.gpsimd.dma_start(out=xk8_sb[:, 6:8, :], in_=r3(xk8T, 6, 2))
            elif pi == 2:
                nc.vector.memset(xv_sb[0:1, :, 0:1], 0)
                for j in range(4):
                    eng = nc.sync if j % 2 == 0 else nc.scalar
                    eng.dma_start(out=xv_sb[:, 4 * j:4 * j + 4, :],
                                  in_=r3(xv_in, 4 * j, 4))
            elif pi == 3:
                nc.vector.memset(wv_sb[0:1, :, 0:1], 0)
                for j in range(2):
                    nc.gpsimd.dma_start(out=wv_sb[:, 4 * j:4 * j + 4, :],
                                        in_=r3(wvT, 4 * j, 4))
            # (pass 4's chains need m cols 768:1024 — lead set3 — so the
            # bulk never competes with a set the current pass is consuming)

        # ---- Phase D: S^T = Xk8^T-slices @ Q'8 via fp8 DoubleRow ----
        # The softmax denominator partials accumulate on the (otherwise
        # idle) vector engine as each exp tile lands, keeping the N=1
        # denominator matmuls off the tensor engine entirely.
        s_part = spool.tile([P, QL], F32, tag="s_part")
        for kt in range(KT):
            acc = ps.tile([P, QL], F32, tag="ps")
            for g in range(NG):
                k_sl = xk8_sb[:, 2 * g:2 * g + 2, kt * P:(kt + 1) * P]
                for c in range(2):
                    nc.tensor.matmul(
                        acc[:, c * 512:(c + 1) * 512], k_sl,
                        q8_sb[:, 2 * g:2 * g + 2, c * 512:(c + 1) * 512],
                        start=(g == 0), stop=(g == NG - 1),
                        perf_mode=DR)
            nc.scalar.activation(pT_sb[:, kt, :], acc, AFT.Exp,
                                 scale=EXP_SCALE)
            if kt == 0:
                nc.vector.tensor_copy(s_part, pT_sb[:, 0, :])
            else:
                nc.vector.tensor_add(s_part, s_part, pT_sb[:, kt, :])
        # finish the partition reduction on gpsimd (off the critical path)
        s_bc = spool.tile([P, QL], F32, tag="s_bc")
        nc.gpsimd.partition_all_reduce(s_bc, s_part, 128,
                                       bass_isa.ReduceOp.add)

        # ---- Phase E: U^T = Xv^T P^T ----
        for db in range(DT):
            acc = ps.tile([P, QL], F32, tag="ps")
            for kt in range(KT):
                v_sl = xv_sb[:, kt, db * P:(db + 1) * P]
                for c in range(2):
                    nc.tensor.matmul(
                        acc[:, c * 512:(c + 1) * 512], v_sl,
                        pT_sb[:, kt, c * 512:(c + 1) * 512],
                        start=(kt == 0), stop=(kt == KT - 1))
            nc.vector.tensor_copy(uT_sb[:, db, :], acc)

        # scatter s_bc's [1,1024] row into per-partition [128,8] layout and
        # take the reciprocal; emitted AFTER phase E so the queue-blocking
        # waits (on the gpsimd reduce) never stall E's copy chain
        s_cols = spool.tile([P, QT], F32, tag="s_cols")
        for t in range(QT):
            eng = nc.sync if t % 2 == 0 else nc.scalar
            eng.dma_start(out=s_cols[:, t:t + 1],
                          in_=s_bc[0:1, t * P:(t + 1) * P])
        r_all = spool.tile([P, QT], F32, tag="r_all")
        nc.vector.reciprocal(r_all, s_cols)

        # ---- Phase F: out = U Wv^T * r ----
        for qt in range(QT):
            acc = ps.tile([P, D], F32, tag="ps")
            for db in range(DT):
                u_sl = uT_sb[:, db, qt * P:(qt + 1) * P]
                for c in range(2):
                    nc.tensor.matmul(
                        acc[:, c * 512:(c + 1) * 512], u_sl,
                        wv_sb[:, db, c * 512:(c + 1) * 512],
                        start=(db == 0), stop=(db == DT - 1))
            # output scaling: one PSUM BANK per engine (vector reads cols
            # 0:512 = bank A, scalar reads 512:1024 = bank B) into separate
            # staging tiles — any finer interleave serializes through the
            # bank-aware cross-engine PSUM-collision tracking
            r_t = r_all[:, qt:qt + 1]
            o_v = o_pool.tile([P, 512], BF, tag="ov", name=f"ov{qt}")
            o_s = o_pool.tile([P, 512], BF, tag="os", name=f"os{qt}")
            nc.vector.tensor_scalar_mul(o_v, acc[:, 0:512], r_t)
            if qt < QT - 1:
                nc.sync.dma_start(out=out[qt * P:(qt + 1) * P, 0:512],
                                  in_=o_v)
            else:
                # last tile: drain latency is exposed — use all 3 rings
                nc.sync.dma_start(out=out[qt * P:(qt + 1) * P, 0:256],
                                  in_=o_v[:, 0:256])
                nc.gpsimd.dma_start(out=out[qt * P:(qt + 1) * P, 256:512],
                                    in_=o_v[:, 256:512])
            nc.scalar.activation(o_s, acc[:, 512:1024], AFT.Copy, scale=r_t)
            nc.scalar.dma_start(out=out[qt * P:(qt + 1) * P, 512:1024],
                                in_=o_s)

    nc.compile()
    return nc


def _get_nc():
    if "nc" not in _CACHE:
        _CACHE["nc"] = _build_nc()
    return _CACHE["nc"]


def make_in_maps(q, k, v, W_q, W_k, W_v):
    bf = ml_dtypes.bfloat16
    f8 = ml_dtypes.float8_e4m3
    W_q = np.asarray(W_q, dtype=np.float32)
    W_k = np.asarray(W_k, dtype=np.float32)
    W_v = np.asarray(W_v, dtype=np.float32)
    m_f32 = (W_q.T @ W_k) * (SM8 / 32.0)
    m_host = m_f32.astype(bf)
    m8_host = m_f32[0:512, :].astype(f8)
    wvT_host = np.ascontiguousarray(W_v.T).astype(bf)
    in_maps = []
    for c in range(8):
        b, h = c // 2, c % 2
        sl = slice(h * 1024, (h + 1) * 1024)
        in_maps.append({
            "m_in": m_host,
            "m8T": m8_host,
            "xqT": np.asarray(q[b, sl, :], dtype=np.float32).T.astype(bf),
            "xq8T": np.ascontiguousarray(
                np.asarray(q[b, sl, :], dtype=np.float32).T[0:512, :]
            ).astype(f8),
            "xk8T": (np.asarray(k[b], dtype=np.float32).T * SK).astype(f8),
            "xv_in": np.asarray(v[b], dtype=np.float32).astype(bf),
            "wvT": wvT_host,
        })
    return in_maps


def kernel(**inputs):
    from concourse import bass_utils

    q = np.asarray(inputs["q_input"], dtype=np.float32)
    k = np.asarray(inputs["k_input"], dtype=np.float32)
    v = np.asarray(inputs["v_input"], dtype=np.float32)

    nc = _get_nc()
    in_maps = make_in_maps(q, k, v, inputs["W_q"], inputs["W_k"], inputs["W_v"])

    res = None
    for attempt in range(3):
        try:
            res = bass_utils.run_bass_kernel_spmd(nc, in_maps,
                                                  core_ids=list(range(8)))
            break
        except Exception:
            if attempt == 2:
                raise

    full = np.empty((4, 2048, 1024), dtype=np.float32)
    for c in range(8):
        b, h = c // 2, c % 2
        full[b, h * 1024:(h + 1) * 1024, :] = np.asarray(
            res.results[c]["out"], dtype=np.float32)
    return full
